# revision 38
# baseline (speedup 1.0000x reference)
"""Trainium2 Bass kernel for nn_DecoderSmoothedMaxPoolingLoss.

Loss (see reference):
  neg  = -log(1 - X)                                    (B,T,K)
  loss = sum_{b, t<len_b, k} neg
         - sum_{b, i in [0,Lw_b), k=tgt_b} neg[b, tau_s_b + i, k]
         + sum_b -log( max_j  clip(conv_same(win_b * valid_b, filt), EPS, 1) * valid_b )
  where tau_s = max(0, w_end + 40 - 60), tau_e = min(tau_s + 60, len),
  Lw = tau_e - tau_s, win_b[i] = X[b, tau_s_b + i, tgt_b].

Sharding: pure data parallel over batch — 8 batches per core on 8 cores.
Each core computes its partial scalar loss on device; host sums the 8
partials (the "all-reduce").

Per core (8 local batches = 12.8 MB), viewed as (128, 25000):
partition p <-> batch b=p//16, t in [250*(p%16), 250*(p%16)+250), k inner.

  bulk:     3 large HWDGE (nc.sync) chunk loads with 40/40/20 KB
            per-partition descriptors (big descriptors amortize the
            per-descriptor latency; HWDGE measured ~25 GB/s/engine).
  neg term: 10 ACT Ln blocks of (128, 2500) (25 t-rows each), each
            writing its per-partition block sum via accum_out — the
            reduction rides the activation pass; DVE never touches the
            bulk data.  Host block mask (-1 = block fully below len_b,
            0 otherwise) dots with the block sums.
  boundary: the partial 25-row block per batch (len_b % 25 != 0) is
            host-sliced into auxA, Ln'd and mask-reduced on device.
  windows:  window rows are host-sliced into auxA (no indirect DMA);
            one-hot select of k=tgt on DVE; exclusion term comes free
            from ACT accum over ln(1 - winv) (invalid lanes give
            ln(1)=0); conv as two small matmuls -> clip/mask/max.
  final:    partial columns -> one matmul with ones -> scalar out.
"""

import numpy as np

import concourse.bass as bass
import concourse.tile as tile
from concourse import bacc
from concourse import mybir
from concourse import bass_utils

AF = mybir.ActivationFunctionType
ALU = mybir.AluOpType
AX = mybir.AxisListType
FP = mybir.dt.float32

B, T, K = 64, 4000, 100
WIN, OFFSET_D, TRUNC, SIGMA = 60, 40, 21, 9
EPS = 1e-8
NCORES = 8
BLOC = B // NCORES          # 8 batches per core
P = 128                     # partitions
FTOT = BLOC * T * K // P    # 25000 free elems per partition
TR = 250                    # t-rows per partition
BR = 25                     # t-rows per ACT block
NBLK = TR // BR             # 10 ACT blocks
FB = BR * K                 # 2500 free elems per block
CHUNK_BLKS = (4, 3, 3)      # DMA chunks in units of ACT blocks

# auxA (8, 2653): valid8 | I8 | m25 | win_raw (host-indexed window
# values X[b, tau_s+i, tgt_b]) | WbH (boundary rows) — all gathers are
# host-side INDEXING (no arithmetic on X leaves the device).  Kept
# small: partitions 0-7 land on only 2 of the 16 DMA engines, so these
# bytes serialize ahead of the bulk chunks there.
O_VAL = 0
O_I8 = O_VAL + WIN
O_M25 = O_I8 + BLOC
O_WR = O_M25 + BR
O_WB = O_WR + WIN
WA = O_WB + FB
WBF = WIN + NBLK            # auxB (128, 70): conv M (rows<60) | Mblk


def _filt_np():
    half = TRUNC // 2
    x = np.arange(-half, half + 1, dtype=np.float32)
    g = np.exp(-0.5 * (x / SIGMA) ** 2).astype(np.float32)
    g = g / g.sum()
    f = np.zeros(WIN, np.float32)
    c = WIN // 2
    f[c - half:c + half + 1] = g
    return f


def _conv_matrix():
    # smoothed[j] = sum_i win[i] * filt[i - j + pl], pl = (WIN-1)//2
    f = _filt_np()
    pl = (WIN - 1) // 2
    idx = np.arange(WIN)
    u = idx[:, None] - idx[None, :] + pl          # (i, j)
    M = np.where((u >= 0) & (u < WIN), f[np.clip(u, 0, WIN - 1)], 0.0)
    return M.astype(np.float32)


_NC_CACHE = None


def _build_program():
    global _NC_CACHE
    if _NC_CACHE is not None:
        return _NC_CACHE

    nc = bacc.Bacc("TRN2", debug=False)
    Xs = nc.dram_tensor("Xs", [BLOC, T, K], FP, kind="ExternalInput").ap()
    auxA = nc.dram_tensor("auxA", [BLOC, WA], FP, kind="ExternalInput").ap()
    auxB = nc.dram_tensor("auxB", [P, WBF], FP, kind="ExternalInput").ap()
    outd = nc.dram_tensor("out", [1, 1], FP, kind="ExternalOutput").ap()

    with tile.TileContext(nc) as tc:
        with tc.tile_pool(name="xin", bufs=1) as xin_pool, \
             tc.tile_pool(name="small", bufs=1) as small, \
             tc.tile_pool(name="psum", bufs=1, space="PSUM") as psum:

            # ---------- HWDGE (sync) queue: auxA (8 descs), the three bulk
            # chunks, then auxB last (128 tiny descs drain after bulk; it is
            # only needed by the late combine). ----------
            auxA_sb = small.tile([BLOC, WA], FP)
            nc.sync.dma_start(out=auxA_sb[:], in_=auxA)
            auxB_sb = small.tile([P, WBF], FP)

            Xp = Xs.rearrange("b t k -> (b t k)").rearrange(
                "(p f) -> p f", p=P)                     # (128, 25000)
            xt = []
            for ci, nb in enumerate(CHUNK_BLKS):
                xt.append(xin_pool.tile([P, nb * FB], FP, tag=f"xb{ci}",
                                        name=f"xb{ci}"))
            base = 0
            for ci, nb in enumerate(CHUNK_BLKS):
                nc.sync.dma_start(out=xt[ci][:], in_=Xp[:, base:base + nb * FB])
                base += nb * FB
            nc.sync.dma_start(out=auxB_sb[:], in_=auxB)

            valid_sl = auxA_sb[0:BLOC, O_VAL:O_VAL + WIN]
            I8_sl = auxA_sb[0:BLOC, O_I8:O_I8 + BLOC]
            m25_sl = auxA_sb[0:BLOC, O_M25:O_M25 + BR]
            win_raw = auxA_sb[0:BLOC, O_WR:O_WR + WIN]
            Wb = auxA_sb[0:BLOC, O_WB:O_WB + FB]
            M_sl = auxB_sb[0:WIN, 0:WIN]
            Mblk_sl = auxB_sb[0:P, WIN:WBF]
            # broadcast AP: m25 over k
            mb2_b = bass.AP(tensor=m25_sl.tensor, offset=m25_sl.offset,
                            ap=[m25_sl.ap[0], m25_sl.ap[1], [0, K]])

            # ---------- result columns ----------
            Cfin = small.tile([P, 4], FP)
            nc.vector.memset(Cfin[:], 0.0)
            scr8 = small.tile([BLOC, FB], FP)

            # ---------- window path (all data in auxA by ~4 us; the whole
            # chain runs under the bulk load) ----------
            winv = small.tile([BLOC, WIN], FP)
            nc.vector.tensor_tensor(out=winv[:], in0=win_raw,
                                    in1=valid_sl, op=ALU.mult)

            # boundary-block Ln (scalar; data ready early; in place)
            nc.scalar.activation(out=Wb, in_=Wb, func=AF.Ln,
                                 bias=1.0, scale=-1.0)
            # exclusion term: ln(1 - winv) is ln(1)=0 on invalid lanes, so
            # one ACT accum gives  sum_i valid * ln(1 - win_raw)  directly.
            lnwv = small.tile([BLOC, WIN], FP)
            nc.scalar.activation(out=lnwv[:], in_=winv[:], func=AF.Ln,
                                 bias=1.0, scale=-1.0,
                                 accum_out=Cfin[0:BLOC, 2:3])

            # smoothed = win_v @ M (transpose first via identity)
            wvt_ps = psum.tile([WIN, BLOC], FP)
            nc.tensor.matmul(out=wvt_ps[:], lhsT=winv[:], rhs=I8_sl,
                             start=True, stop=True)
            wvt = small.tile([WIN, BLOC], FP)
            nc.vector.tensor_copy(out=wvt[:], in_=wvt_ps[:])
            sm_ps = psum.tile([BLOC, WIN], FP)
            nc.tensor.matmul(out=sm_ps[:], lhsT=wvt[:], rhs=M_sl,
                             start=True, stop=True)
            smc = small.tile([BLOC, WIN], FP)
            nc.vector.tensor_scalar(out=smc[:], in0=sm_ps[:],
                                    scalar1=EPS, scalar2=1.0,
                                    op0=ALU.max, op1=ALU.min)
            smv = small.tile([BLOC, WIN], FP)
            nc.vector.tensor_tensor(out=smv[:], in0=smc[:], in1=valid_sl,
                                    op=ALU.mult)
            mx = small.tile([BLOC, 1], FP)
            nc.vector.tensor_reduce(out=mx[:], in_=smv[:], axis=AX.X,
                                    op=ALU.max)
            lnmx = small.tile([BLOC, 1], FP)
            nc.scalar.activation(out=lnmx[:], in_=mx[:], func=AF.Ln)
            nc.vector.tensor_scalar_mul(Cfin[0:BLOC, 3:4], lnmx[:], -1.0)

            # ---------- neg term: 10 ACT Ln blocks w/ accum ----------
            AC = small.tile([P, NBLK], FP)
            g = 0
            for c, nb in enumerate(CHUNK_BLKS):
                for l in range(nb):
                    sl = xt[c][:, l * FB:(l + 1) * FB]
                    nc.scalar.activation(
                        out=sl, in_=sl, func=AF.Ln, bias=1.0, scale=-1.0,
                        accum_out=AC[:, g:g + 1])
                    g += 1

            # boundary-block DVE reduce (inputs ready early; emitted late so
            # the DVE-counter chain never gates hoisted scalar ops)
            nc.vector.tensor_tensor(
                out=scr8[:].rearrange("b (r k) -> b r k", k=K),
                in0=Wb.rearrange("b (r k) -> b r k", k=K),
                in1=mb2_b, op=ALU.mult)
            nc.vector.tensor_reduce(out=Cfin[0:BLOC, 1:2], in_=scr8[:],
                                    axis=AX.X, op=ALU.add)

            # dot block sums with block mask
            scrA = small.tile([P, NBLK], FP)
            nc.vector.tensor_tensor(out=scrA[:], in0=AC[:], in1=Mblk_sl,
                                    op=ALU.mult)
            nc.vector.tensor_reduce(out=Cfin[0:P, 0:1], in_=scrA[:],
                                    axis=AX.X, op=ALU.add)

            # ---------- final partition reduce ----------
            ones = small.tile([P, 1], FP)
            nc.vector.memset(ones[:], 1.0)
            tot_ps = psum.tile([1, 4], FP)
            nc.tensor.matmul(out=tot_ps[:], lhsT=ones[:], rhs=Cfin[:],
                             start=True, stop=True)
            tot = small.tile([1, 1], FP)
            nc.vector.tensor_reduce(out=tot[:], in_=tot_ps[:], axis=AX.X,
                                    op=ALU.add)
            nc.gpsimd.dma_start(out=outd, in_=tot[:])

    nc.compile()
    _NC_CACHE = nc
    return nc


def _make_in_maps(X, lengths, tgt, w_end):
    X = np.ascontiguousarray(np.asarray(X, dtype=np.float32))
    lengths = np.asarray(lengths, dtype=np.int64)
    tgt = np.asarray(tgt, dtype=np.int64)
    w_end = np.asarray(w_end, dtype=np.int64)

    tau_s = np.maximum(0, w_end + OFFSET_D - WIN)
    tau_e = np.minimum(tau_s + WIN, lengths)
    Lw = tau_e - tau_s

    Mmat = _conv_matrix()
    I8 = np.eye(BLOC, dtype=np.float32)

    in_maps = []
    for cr in range(NCORES):
        bs = slice(cr * BLOC, (cr + 1) * BLOC)
        ls, ts, lw, tg = lengths[bs], tau_s[bs], Lw[bs], tgt[bs]
        Xc = X[bs]

        valid8 = (np.arange(WIN)[None, :] < lw[:, None]).astype(np.float32)

        # boundary block: rows [25*floor(len/25), len) when len % 25 != 0
        bstart = (ls // BR) * BR
        rstar = ls - bstart                       # 0..24
        m25 = -(np.arange(BR)[None, :] < rstar[:, None]).astype(np.float32)

        # host-side INDEXING (no arithmetic on X): window values at k=tgt
        # and the raw boundary rows
        wrH = np.stack([Xc[b, ts[b]:ts[b] + WIN, tg[b]]
                        for b in range(BLOC)])                   # (8, 60)
        bsrc = np.where(rstar > 0, bstart, 0)
        WbH = np.stack([Xc[b, bsrc[b]:bsrc[b] + BR, :].reshape(-1)
                        for b in range(BLOC)])                   # (8, 2500)

        auxA = np.concatenate([valid8, I8, m25, wrH, WbH], axis=1)

        # Mblk[p, g] = -1 iff block [250*(p%16)+25g, +25) fully < len_b
        pidx = np.arange(P)
        qq = (pidx % (P // BLOC)) * TR            # 250*(p%16)
        bb = pidx // (P // BLOC)                  # batch of partition
        gblk = np.arange(NBLK)
        blk_end = qq[:, None] + BR * (gblk[None, :] + 1)
        Mblk = -(blk_end <= ls[bb][:, None]).astype(np.float32)  # (128, 10)

        Mpad = np.zeros((P, WIN), np.float32)
        Mpad[0:WIN] = Mmat
        auxB = np.concatenate([Mpad, Mblk], axis=1)              # (128, 70)

        in_maps.append({
            "Xs": np.ascontiguousarray(Xc),
            "auxA": np.ascontiguousarray(auxA),
            "auxB": np.ascontiguousarray(auxB),
        })
    return in_maps


def kernel(X, lengths, tgt, w_end):
    nc = _build_program()
    in_maps = _make_in_maps(X, lengths, tgt, w_end)
    res = bass_utils.run_bass_kernel_spmd(
        nc, in_maps, core_ids=list(range(NCORES)))
    total = np.float32(0.0)
    for c in range(NCORES):
        total += np.float32(res.results[c]["out"][0, 0])
    return np.array(total, dtype=np.float32)


# revision 39
# speedup vs baseline: 1.1060x; 1.1060x over previous
"""Trainium2 Bass kernel for nn_DecoderSmoothedMaxPoolingLoss.

Loss (see reference):
  neg  = -log(1 - X)                                    (B,T,K)
  loss = sum_{b, t<len_b, k} neg
         - sum_{b, i in [0,Lw_b), k=tgt_b} neg[b, tau_s_b + i, k]
         + sum_b -log( max_j  clip(conv_same(win_b * valid_b, filt), EPS, 1) * valid_b )
  where tau_s = max(0, w_end + 40 - 60), tau_e = min(tau_s + 60, len),
  Lw = tau_e - tau_s, win_b[i] = X[b, tau_s_b + i, tgt_b].

Sharding: pure data parallel over batch — 8 batches per core on 8 cores.
Each core computes its partial scalar loss on device; host sums the 8
partials (the "all-reduce").

Per core (8 local batches = 12.8 MB), viewed as (128, 25000):
partition p <-> batch b=p//16, t in [250*(p%16), 250*(p%16)+250), k inner.

  bulk:     3 large HWDGE (nc.sync) chunk loads with 40/40/20 KB
            per-partition descriptors (big descriptors amortize the
            per-descriptor latency; HWDGE measured ~25 GB/s/engine).
  neg term: 10 ACT Ln blocks of (128, 2500) (25 t-rows each), each
            writing its per-partition block sum via accum_out — the
            reduction rides the activation pass; DVE never touches the
            bulk data.  Host block mask (-1 = block fully below len_b,
            0 otherwise) dots with the block sums.
  boundary: the partial 25-row block per batch (len_b % 25 != 0) is
            host-sliced into auxA, Ln'd and mask-reduced on device.
  windows:  window rows are host-sliced into auxA (no indirect DMA);
            one-hot select of k=tgt on DVE; exclusion term comes free
            from ACT accum over ln(1 - winv) (invalid lanes give
            ln(1)=0); conv as two small matmuls -> clip/mask/max.
  final:    partial columns -> one matmul with ones -> scalar out.
"""

import numpy as np

import concourse.bass as bass
import concourse.tile as tile
from concourse import bacc
from concourse import mybir
from concourse import bass_utils

AF = mybir.ActivationFunctionType
ALU = mybir.AluOpType
AX = mybir.AxisListType
FP = mybir.dt.float32

B, T, K = 64, 4000, 100
WIN, OFFSET_D, TRUNC, SIGMA = 60, 40, 21, 9
EPS = 1e-8
NCORES = 8
BLOC = B // NCORES          # 8 batches per core
P = 128                     # partitions
FTOT = BLOC * T * K // P    # 25000 free elems per partition
TR = 250                    # t-rows per partition
BR = 25                     # t-rows per ACT block
NBLK = TR // BR             # 10 ACT blocks
FB = BR * K                 # 2500 free elems per block
CHUNK_BLKS = (4, 4, 2)      # DMA chunks in units of ACT blocks

# auxA (8, 2653): valid8 | I8 | m25 | win_raw (host-indexed window
# values X[b, tau_s+i, tgt_b]) | WbH (boundary rows) — all gathers are
# host-side INDEXING (no arithmetic on X leaves the device).  Kept
# small: partitions 0-7 land on only 2 of the 16 DMA engines, so these
# bytes serialize ahead of the bulk chunks there.
O_VAL = 0
O_I8 = O_VAL + WIN
O_M25 = O_I8 + BLOC
O_WR = O_M25 + BR
O_WB = O_WR + WIN
WA = O_WB + FB
WBF = WIN + NBLK            # auxB (128, 70): conv M (rows<60) | Mblk


def _filt_np():
    half = TRUNC // 2
    x = np.arange(-half, half + 1, dtype=np.float32)
    g = np.exp(-0.5 * (x / SIGMA) ** 2).astype(np.float32)
    g = g / g.sum()
    f = np.zeros(WIN, np.float32)
    c = WIN // 2
    f[c - half:c + half + 1] = g
    return f


def _conv_matrix():
    # smoothed[j] = sum_i win[i] * filt[i - j + pl], pl = (WIN-1)//2
    f = _filt_np()
    pl = (WIN - 1) // 2
    idx = np.arange(WIN)
    u = idx[:, None] - idx[None, :] + pl          # (i, j)
    M = np.where((u >= 0) & (u < WIN), f[np.clip(u, 0, WIN - 1)], 0.0)
    return M.astype(np.float32)


_NC_CACHE = None


def _build_program():
    global _NC_CACHE
    if _NC_CACHE is not None:
        return _NC_CACHE

    nc = bacc.Bacc("TRN2", debug=False)
    Xs = nc.dram_tensor("Xs", [BLOC, T, K], FP, kind="ExternalInput").ap()
    auxA = nc.dram_tensor("auxA", [BLOC, WA], FP, kind="ExternalInput").ap()
    auxB = nc.dram_tensor("auxB", [P, WBF], FP, kind="ExternalInput").ap()
    outd = nc.dram_tensor("out", [1, 1], FP, kind="ExternalOutput").ap()

    with tile.TileContext(nc) as tc:
        with tc.tile_pool(name="xin", bufs=1) as xin_pool, \
             tc.tile_pool(name="small", bufs=1) as small, \
             tc.tile_pool(name="psum", bufs=1, space="PSUM") as psum:

            # ---------- HWDGE (sync) queue: auxA (8 descs), the three bulk
            # chunks, then auxB last (128 tiny descs drain after bulk; it is
            # only needed by the late combine). ----------
            auxA_sb = small.tile([BLOC, WA], FP)
            nc.sync.dma_start(out=auxA_sb[:], in_=auxA)
            auxB_sb = small.tile([P, WBF], FP)

            Xp = Xs.rearrange("b t k -> (b t k)").rearrange(
                "(p f) -> p f", p=P)                     # (128, 25000)
            xt = []
            for ci, nb in enumerate(CHUNK_BLKS):
                xt.append(xin_pool.tile([P, nb * FB], FP, tag=f"xb{ci}",
                                        name=f"xb{ci}"))
            base = 0
            for ci, nb in enumerate(CHUNK_BLKS):
                nc.sync.dma_start(out=xt[ci][:], in_=Xp[:, base:base + nb * FB])
                base += nb * FB
            nc.sync.dma_start(out=auxB_sb[:], in_=auxB)

            valid_sl = auxA_sb[0:BLOC, O_VAL:O_VAL + WIN]
            I8_sl = auxA_sb[0:BLOC, O_I8:O_I8 + BLOC]
            m25_sl = auxA_sb[0:BLOC, O_M25:O_M25 + BR]
            win_raw = auxA_sb[0:BLOC, O_WR:O_WR + WIN]
            Wb = auxA_sb[0:BLOC, O_WB:O_WB + FB]
            M_sl = auxB_sb[0:WIN, 0:WIN]
            Mblk_sl = auxB_sb[0:P, WIN:WBF]
            # broadcast AP: m25 over k
            mb2_b = bass.AP(tensor=m25_sl.tensor, offset=m25_sl.offset,
                            ap=[m25_sl.ap[0], m25_sl.ap[1], [0, K]])

            # ---------- result columns ----------
            Cfin = small.tile([P, 4], FP)
            nc.vector.memset(Cfin[:], 0.0)
            scr8 = small.tile([BLOC, FB], FP)

            # ---------- window path (all data in auxA by ~4 us; the whole
            # chain runs under the bulk load) ----------
            winv = small.tile([BLOC, WIN], FP)
            nc.vector.tensor_tensor(out=winv[:], in0=win_raw,
                                    in1=valid_sl, op=ALU.mult)

            # boundary-block Ln (scalar; data ready early; in place)
            nc.scalar.activation(out=Wb, in_=Wb, func=AF.Ln,
                                 bias=1.0, scale=-1.0)
            # exclusion term: ln(1 - winv) is ln(1)=0 on invalid lanes, so
            # one ACT accum gives  sum_i valid * ln(1 - win_raw)  directly.
            lnwv = small.tile([BLOC, WIN], FP)
            nc.scalar.activation(out=lnwv[:], in_=winv[:], func=AF.Ln,
                                 bias=1.0, scale=-1.0,
                                 accum_out=Cfin[0:BLOC, 2:3])

            # smoothed = win_v @ M (transpose first via identity)
            wvt_ps = psum.tile([WIN, BLOC], FP)
            nc.tensor.matmul(out=wvt_ps[:], lhsT=winv[:], rhs=I8_sl,
                             start=True, stop=True)
            wvt = small.tile([WIN, BLOC], FP)
            nc.vector.tensor_copy(out=wvt[:], in_=wvt_ps[:])
            sm_ps = psum.tile([BLOC, WIN], FP)
            nc.tensor.matmul(out=sm_ps[:], lhsT=wvt[:], rhs=M_sl,
                             start=True, stop=True)
            smc = small.tile([BLOC, WIN], FP)
            nc.vector.tensor_scalar(out=smc[:], in0=sm_ps[:],
                                    scalar1=EPS, scalar2=1.0,
                                    op0=ALU.max, op1=ALU.min)
            smv = small.tile([BLOC, WIN], FP)
            nc.vector.tensor_tensor(out=smv[:], in0=smc[:], in1=valid_sl,
                                    op=ALU.mult)
            mx = small.tile([BLOC, 1], FP)
            nc.vector.tensor_reduce(out=mx[:], in_=smv[:], axis=AX.X,
                                    op=ALU.max)
            lnmx = small.tile([BLOC, 1], FP)
            nc.scalar.activation(out=lnmx[:], in_=mx[:], func=AF.Ln)
            nc.vector.tensor_scalar_mul(Cfin[0:BLOC, 3:4], lnmx[:], -1.0)

            # ---------- neg term: 10 ACT Ln blocks w/ accum ----------
            AC = small.tile([P, NBLK], FP)
            g = 0
            for c, nb in enumerate(CHUNK_BLKS):
                for l in range(nb):
                    sl = xt[c][:, l * FB:(l + 1) * FB]
                    nc.scalar.activation(
                        out=sl, in_=sl, func=AF.Ln, bias=1.0, scale=-1.0,
                        accum_out=AC[:, g:g + 1])
                    g += 1

            # boundary-block DVE reduce (inputs ready early; emitted late so
            # the DVE-counter chain never gates hoisted scalar ops)
            nc.vector.tensor_tensor(
                out=scr8[:].rearrange("b (r k) -> b r k", k=K),
                in0=Wb.rearrange("b (r k) -> b r k", k=K),
                in1=mb2_b, op=ALU.mult)
            nc.vector.tensor_reduce(out=Cfin[0:BLOC, 1:2], in_=scr8[:],
                                    axis=AX.X, op=ALU.add)

            # dot block sums with block mask
            scrA = small.tile([P, NBLK], FP)
            nc.vector.tensor_tensor(out=scrA[:], in0=AC[:], in1=Mblk_sl,
                                    op=ALU.mult)
            nc.vector.tensor_reduce(out=Cfin[0:P, 0:1], in_=scrA[:],
                                    axis=AX.X, op=ALU.add)

            # ---------- final partition reduce ----------
            ones = small.tile([P, 1], FP)
            nc.vector.memset(ones[:], 1.0)
            tot_ps = psum.tile([1, 4], FP)
            nc.tensor.matmul(out=tot_ps[:], lhsT=ones[:], rhs=Cfin[:],
                             start=True, stop=True)
            tot = small.tile([1, 1], FP)
            nc.vector.tensor_reduce(out=tot[:], in_=tot_ps[:], axis=AX.X,
                                    op=ALU.add)
            nc.gpsimd.dma_start(out=outd, in_=tot[:])

    nc.compile()
    _NC_CACHE = nc
    return nc


def _make_in_maps(X, lengths, tgt, w_end):
    X = np.ascontiguousarray(np.asarray(X, dtype=np.float32))
    lengths = np.asarray(lengths, dtype=np.int64)
    tgt = np.asarray(tgt, dtype=np.int64)
    w_end = np.asarray(w_end, dtype=np.int64)

    tau_s = np.maximum(0, w_end + OFFSET_D - WIN)
    tau_e = np.minimum(tau_s + WIN, lengths)
    Lw = tau_e - tau_s

    Mmat = _conv_matrix()
    I8 = np.eye(BLOC, dtype=np.float32)

    in_maps = []
    for cr in range(NCORES):
        bs = slice(cr * BLOC, (cr + 1) * BLOC)
        ls, ts, lw, tg = lengths[bs], tau_s[bs], Lw[bs], tgt[bs]
        Xc = X[bs]

        valid8 = (np.arange(WIN)[None, :] < lw[:, None]).astype(np.float32)

        # boundary block: rows [25*floor(len/25), len) when len % 25 != 0
        bstart = (ls // BR) * BR
        rstar = ls - bstart                       # 0..24
        m25 = -(np.arange(BR)[None, :] < rstar[:, None]).astype(np.float32)

        # host-side INDEXING (no arithmetic on X): window values at k=tgt
        # and the raw boundary rows
        wrH = np.stack([Xc[b, ts[b]:ts[b] + WIN, tg[b]]
                        for b in range(BLOC)])                   # (8, 60)
        bsrc = np.where(rstar > 0, bstart, 0)
        WbH = np.stack([Xc[b, bsrc[b]:bsrc[b] + BR, :].reshape(-1)
                        for b in range(BLOC)])                   # (8, 2500)

        auxA = np.concatenate([valid8, I8, m25, wrH, WbH], axis=1)

        # Mblk[p, g] = -1 iff block [250*(p%16)+25g, +25) fully < len_b
        pidx = np.arange(P)
        qq = (pidx % (P // BLOC)) * TR            # 250*(p%16)
        bb = pidx // (P // BLOC)                  # batch of partition
        gblk = np.arange(NBLK)
        blk_end = qq[:, None] + BR * (gblk[None, :] + 1)
        Mblk = -(blk_end <= ls[bb][:, None]).astype(np.float32)  # (128, 10)

        Mpad = np.zeros((P, WIN), np.float32)
        Mpad[0:WIN] = Mmat
        auxB = np.concatenate([Mpad, Mblk], axis=1)              # (128, 70)

        in_maps.append({
            "Xs": np.ascontiguousarray(Xc),
            "auxA": np.ascontiguousarray(auxA),
            "auxB": np.ascontiguousarray(auxB),
        })
    return in_maps


def kernel(X, lengths, tgt, w_end):
    nc = _build_program()
    in_maps = _make_in_maps(X, lengths, tgt, w_end)
    res = bass_utils.run_bass_kernel_spmd(
        nc, in_maps, core_ids=list(range(NCORES)))
    total = np.float32(0.0)
    for c in range(NCORES):
        total += np.float32(res.results[c]["out"][0, 0])
    return np.array(total, dtype=np.float32)
